# revision 34
# baseline (speedup 1.0000x reference)
"""ArcFace loss (margin softmax CE) on 8 TRN2 NeuronCores.

Strategy (model-parallel softmax CE, classes sharded over 8 cores):
  - host: shard W row-wise by class (12500/core, zero-padded to 12544),
    transpose to [512, Cp] and cast bf16; gather wl = w[labels] (layout
    prep only - all math runs on device).
  - device (SPMD, identical graph on all cores):
      * normalize feats; build fnT (d-major) via PE transposes.
      * per-class weight norms via ones-matmul over squared wT chunks,
        pipelined in groups of 5 chunks; rsqrt computed on the
        VectorEngine (bit-trick seed + Newton) so ScalarE runs ONLY
        Exp in the hot region (a single ACT table set - HW reloads
        tables on every function-set switch, ~2.7us each).
      * main: Z[n,c-chunk] = fnT.T @ wnT (bf16, PSUM f32); fused
        ACT exp(S*z - 64) with accum_out giving per-row partial
        softmax sums (fixed max 64 = S*max|cos| keeps all terms <= 1).
      * label-margin path computed redundantly on every core from wl.
      * AllReduce(add) the [1024] partial sums; each core finishes:
        loss = mean(64 + ln(P + delta) - S*t), with Ln rescaled by an
        exact 2^70 (HW Ln is inaccurate for ~1e-21 args).
"""

import math

import numpy as np
import ml_dtypes

import concourse.bass as bass
import concourse.tile as tile
from concourse import bacc, mybir
from concourse.bass import ts, ds
from concourse.bass_utils import run_bass_kernel_spmd
from concourse.masks import make_identity

FP = mybir.dt.float32
BF = mybir.dt.bfloat16
I32 = mybir.dt.int32
AF = mybir.ActivationFunctionType
OP = mybir.AluOpType

# problem constants (hardcoded per harness contract)
MARGIN = 0.5
S = 64.0
COS_M = math.cos(MARGIN)
SIN_M = math.sin(MARGIN)
MIN_COS = math.cos(math.pi - MARGIN)
C = 100000
D = 512
N = 1024
NCORES = 8
CS = C // NCORES          # 12500 classes per core
CP = 12544                # padded: 98 * 128
NT = N // 128             # 8 row tiles
FB = -64.0                # fixed log-domain shift (= -S * max cos)
LN2 = math.log(2.0)
RSQRT_MAGIC = float(0x5F3759DF)

# W column groups (separate SBUF tiles -> groups pipeline freely);
# norm sub-chunks of 512 and main exp chunks of up to 1024 within a group
NGRP = 6
GRPOFF = [0, 1024, 3584, 6144, 8704, 11264]
GRPSZ = [1024, 2560, 2560, 2560, 2560, 1280]
def _subchunks(gsz):
    out, o = [], 0
    while o < gsz:
        s = min(512, gsz - o)
        out.append((o, s))
        o += s
    return out
def _mainchunks(gsz):
    out, o = [], 0
    while o < gsz:
        s = min(1024, gsz - o)
        out.append((o, s))
        o += s
    return out
SUBCH = [_subchunks(s) for s in GRPSZ]
MAINCH = [_mainchunks(s) for s in GRPSZ]
NCH = sum(len(m) for m in MAINCH)   # 15 exp chunks total


def dve_rsqrt(nc, pool, x_ap, rows, cols, prefix, iters=2):
    """1/sqrt(x) on the VectorEngine only: quake-style bit seed via a
    float round-trip (no int multiply needed), then Newton iterations.
    x > 0, f32. Returns the result AP ([rows, cols] f32)."""
    xs = pool.tile([rows, cols], I32, tag=f"{prefix}_xs", name=f"{prefix}_xs")
    nc.vector.tensor_single_scalar(xs[:], x_ap.bitcast(I32), 1,
                                   OP.logical_shift_right)
    xf = pool.tile([rows, cols], FP, tag=f"{prefix}_xf", name=f"{prefix}_xf")
    nc.vector.tensor_copy(xf[:], xs[:])
    yf = pool.tile([rows, cols], FP, tag=f"{prefix}_yf", name=f"{prefix}_yf")
    nc.vector.tensor_scalar(yf[:], xf[:], -1.0, RSQRT_MAGIC, OP.mult, OP.add)
    yi = pool.tile([rows, cols], I32, tag=f"{prefix}_yi", name=f"{prefix}_yi")
    nc.vector.tensor_copy(yi[:], yf[:])
    y = yi[:].bitcast(FP)
    for it in range(iters):
        t1 = pool.tile([rows, cols], FP, tag=f"{prefix}_t1{it}",
                       name=f"{prefix}_t1_{it}")
        nc.vector.tensor_mul(t1[:], y, y)
        nc.vector.tensor_mul(t1[:], t1[:], x_ap)
        nc.vector.tensor_scalar(t1[:], t1[:], -0.5, 1.5, OP.mult, OP.add)
        yn = pool.tile([rows, cols], FP, tag=f"{prefix}_yn{it}",
                       name=f"{prefix}_yn_{it}")
        nc.vector.tensor_mul(yn[:], y, t1[:])
        y = yn[:]
    return y


def build_nc():
    nc = bacc.Bacc(
        "TRN2",
        target_bir_lowering=False,
        debug=False,
        enable_asserts=False,
        num_devices=NCORES,
    )

    # activation-bias constants must be pre-registered as const APs
    for val in (FB,):
        t = nc.alloc_sbuf_tensor(f"const-f32-{val}", [128, 1], FP)
        nc.gpsimd.memset(t.ap(), val)
        nc.const_aps.aps[(FP, val)] = t.ap()
    nc.all_engine_barrier()

    wt_d = nc.dram_tensor("wt", [D, CP], BF, kind="ExternalInput")
    feats_d = nc.dram_tensor("feats", [N, D], FP, kind="ExternalInput")
    wl_d = nc.dram_tensor("wl", [N, D], FP, kind="ExternalInput")
    out_d = nc.dram_tensor("out", [1, 1], FP, kind="ExternalOutput")

    n2_d = nc.dram_tensor("n2scratch", [1, CP], FP)
    inv_d = nc.dram_tensor("invscratch", [1, CP], BF)
    # consistent (p, t) layout on every core; AllReduce is elementwise
    cc_in = nc.dram_tensor("cc_in", [128, NT], FP)
    cc_out = nc.dram_tensor("cc_out", [128, NT], FP)
    ccw_in = nc.dram_tensor("ccw_in", [1, 32], FP)
    ccw_out = nc.dram_tensor("ccw_out", [1, 32], FP)

    # [128, 98] strided views of the per-class scratch vectors (c = g*128+p)
    n2_grid = n2_d.ap().rearrange("a (g p) -> (a p) g", p=128)
    inv_grid = inv_d.ap().rearrange("a (g p) -> (a p) g", p=128)

    with tile.TileContext(nc) as tc, (
        tc.tile_pool(name="const", bufs=1)
    ) as constp, (
        tc.tile_pool(name="wres", bufs=1)
    ) as wres, (
        tc.tile_pool(name="fres", bufs=1)
    ) as fres, (
        tc.tile_pool(name="small", bufs=1)
    ) as small, (
        tc.tile_pool(name="work", bufs=3)
    ) as work, (
        tc.tile_pool(name="msmall", bufs=1)
    ) as msmall, (
        tc.tile_pool(name="zpsum", bufs=3, space="PSUM")
    ) as zpsum, (
        tc.tile_pool(name="psum0", bufs=1, space="PSUM")
    ) as psum0, (
        tc.tile_pool(name="psumn2", bufs=2, space="PSUM")
    ) as psumn2:
        identity = constp.tile([128, 128], BF, tag="identity")
        make_identity(nc, identity[:])
        ones_bf = constp.tile([128, 1], BF, tag="ones_bf")
        nc.vector.memset(ones_bf[:], 1.0)

        # small input loads first so they don't queue behind 12.8MB of W
        fts = []
        for t in range(NT):
            f_t = fres.tile([128, D], FP, tag=f"f_{t}", name=f"f_{t}")
            fts.append(f_t)
            nc.sync.dma_start(out=f_t[:], in_=feats_d.ap()[ts(t, 128), :])
        wlts = []
        for t in range(NT):
            wl_t = fres.tile([128, D], FP, tag=f"wl_{t}", name=f"wl_{t}")
            wlts.append(wl_t)
            nc.sync.dma_start(out=wl_t[:], in_=wl_d.ap()[ts(t, 128), :])

        # ---- W load: per (d-chunk, group) tiles so each group's
        #      normalize/read pipeline is independent ----
        wsb = [[None] * NGRP for _ in range(4)]
        for g in range(NGRP):
            for j in range(4):
                wj = wres.tile([128, GRPSZ[g]], BF, tag=f"wsb{j}_{g}",
                               name=f"wsb{j}_{g}")
                wsb[j][g] = wj
                nc.sync.dma_start(
                    out=wj[:],
                    in_=wt_d.ap()[ts(j, 128), ds(GRPOFF[g], GRPSZ[g])],
                )

        # ---- feats prep: batched row norms, fnT via PE transpose ----
        ssq = small.tile([128, NT], FP, tag="ssq")
        dump = work.tile([128, D], FP, tag="dump", bufs=4)
        for t in range(NT):
            nc.vector.scalar_tensor_tensor(
                out=dump[:], in0=fts[t][:], scalar=1.0, in1=fts[t][:],
                op0=OP.mult, op1=OP.mult,
                accum_out=ssq[:, ts(t, 1)],
            )
        inv_f = dve_rsqrt(nc, small, ssq[:], 128, NT, "rsf", iters=2)

        fn32 = []   # normalized feats, f32, natural layout (label path)
        fnT = [
            fres.tile([128, N], BF, tag=f"fnT{j}", name=f"fnT{j}")
            for j in range(4)
        ]
        for t in range(NT):
            fn_t = fres.tile([128, D], FP, tag=f"fn32_{t}", name=f"fn32_{t}")
            fn32.append(fn_t)
            nc.scalar.mul(fn_t[:], fts[t][:], inv_f[:, ts(t, 1)])
            fnb_t = work.tile([128, D], BF, tag="fnb_t")
            nc.scalar.mul(fnb_t[:], fts[t][:], inv_f[:, ts(t, 1)])
            for j in range(4):
                tp = psum0.tile([128, 128], BF, tag="tp")
                nc.tensor.transpose(tp[:], fnb_t[:, ts(j, 128)], identity[:])
                nc.vector.tensor_copy(fnT[j][:, ts(t, 128)], tp[:])

        # ---- pipelined: weight norms + normalize per group, then the
        #      main matmuls for that group's chunks (chunk-outer, t-inner)
        pall = msmall.tile([128, NT], FP, tag="pall")
        rows = [
            msmall.tile([128, NCH], FP, tag=f"rows{t}", name=f"rows{t}")
            for t in range(NT)
        ]
        def emit_norm_block(g):
            gall, gsz = GRPOFF[g], GRPSZ[g]
            # n2[c] = sum_d wT[d,c]^2 via ones-matmul over squared chunks
            for c0, csz in SUBCH[g]:
                n2p = psumn2.tile([1, 512], FP, tag="n2p", bufs=1,
                                  name=f"n2p_{g}_{c0}")
                for j in range(4):
                    wsq = work.tile([128, 512], BF, tag="wsq",
                                    name=f"wsq_{g}_{c0}_{j}")
                    nc.vector.tensor_mul(wsq[:, :csz],
                                         wsb[j][g][:, ds(c0, csz)],
                                         wsb[j][g][:, ds(c0, csz)])
                    nc.tensor.matmul(
                        n2p[:, :csz], ones_bf[:], wsq[:, :csz],
                        start=(j == 0), stop=(j == 3),
                    )
                n2c = work.tile([1, 512], FP, tag="n2c",
                                name=f"n2c_{g}_{c0}")
                nc.vector.tensor_copy(n2c[:, :csz], n2p[:, :csz])
                nc.gpsimd.dma_start(out=n2_d.ap()[:, ds(gall + c0, csz)],
                                    in_=n2c[:, :csz])

            # inv = rsqrt(n2 + eps) in [128, G] grid layout (DVE only)
            gcols = gsz // 128
            gw0 = gall // 128
            n2g = work.tile([128, 20], FP, tag="n2g", name=f"n2g_{g}")
            nc.gpsimd.dma_start(out=n2g[:, :gcols],
                                in_=n2_grid[:, ds(gw0, gcols)])
            # guard pad columns (n2 == 0): add 1e-24 so rsqrt stays finite
            nc.vector.tensor_scalar_add(n2g[:, :gcols], n2g[:, :gcols], 1e-24)
            invw = dve_rsqrt(nc, work, n2g[:, :gcols], 128, gcols,
                             "rsg", iters=1)
            invgb = work.tile([128, 20], BF, tag="invgb", name=f"invgb_{g}")
            nc.vector.tensor_copy(invgb[:, :gcols], invw)
            nc.gpsimd.dma_start(out=inv_grid[:, ds(gw0, gcols)],
                                in_=invgb[:, :gcols])

            # normalize resident W for the whole group (broadcast inv)
            invbg = work.tile([128, 2560], BF, tag="invbg",
                              name=f"invbg_{g}")
            nc.gpsimd.dma_start(
                out=invbg[:, :gsz],
                in_=inv_d.ap()[:, ds(gall, gsz)].broadcast_to([128, gsz]),
            )
            for j in range(4):
                nc.vector.tensor_mul(wsb[j][g][:], wsb[j][g][:],
                                     invbg[:, :gsz])

        def emit_main(g, chbase):
            # main: Z = fnT.T @ wnT, exp chunks up to 1024 wide
            for ci, (c0, csz) in enumerate(MAINCH[g]):
                ch = chbase + ci
                for t in range(NT):
                    z = zpsum.tile([128, 1024], FP, tag="z",
                                   name=f"z_{g}_{ci}_{t}")
                    for h0 in range(0, csz, 512):
                        hsz = min(512, csz - h0)
                        for j in range(4):
                            nc.tensor.matmul(
                                z[:, ds(h0, hsz)],
                                fnT[j][:, ts(t, 128)],
                                wsb[j][g][:, ds(c0 + h0, hsz)],
                                start=(j == 0), stop=(j == 3),
                            )
                    pd = work.tile([128, 1024], BF, tag="pd",
                                   name=f"pd_{g}_{ci}_{t}")
                    nc.scalar.activation(
                        pd[:, :csz], z[:, :csz], AF.Exp,
                        bias=FB, scale=S,
                        accum_out=rows[t][:, ts(ch, 1)],
                    )
            return chbase + len(MAINCH[g])

        # pipelined emission: PE is in-order, so each group's norm
        # matmuls must be issued BEFORE the previous group's mains or
        # the inv round-trip latency lands on the PE critical path
        emit_norm_block(0)
        emit_norm_block(1)
        chbase = 0
        for g in range(NGRP):
            if g + 2 <= NGRP - 1:
                emit_norm_block(g + 2)
            if g + 2 == NGRP:
                # warm up the collective path (first collective pays a
                # large ncfw setup cost ~40us; this one overlaps the
                # last main blocks so the real one at the end is ~20us)
                warm = constp.tile([1, 32], FP, tag="warm")
                nc.vector.memset(warm[:], 0.0)
                nc.sync.dma_start(out=ccw_in.ap(), in_=warm[:])
                nc.gpsimd.collective_compute(
                    "AllReduce",
                    OP.add,
                    replica_groups=[list(range(NCORES))],
                    ins=[ccw_in.ap().opt()],
                    outs=[ccw_out.ap().opt()],
                )
            chbase = emit_main(g, chbase)

        for t in range(NT):
            nc.vector.tensor_reduce(
                pall[:, ts(t, 1)], rows[t][:],
                axis=mybir.AxisListType.X, op=OP.add,
            )

        # ---- label-margin path (redundant on every core) ----
        cosl = small.tile([128, NT], FP, tag="cosl")
        wsql = small.tile([128, NT], FP, tag="wsql")
        for t in range(NT):
            nc.vector.scalar_tensor_tensor(
                out=dump[:], in0=wlts[t][:], scalar=1.0, in1=wlts[t][:],
                op0=OP.mult, op1=OP.mult,
                accum_out=wsql[:, ts(t, 1)],
            )
        winv = dve_rsqrt(nc, small, wsql[:], 128, NT, "rsw", iters=2)
        for t in range(NT):
            wln_t = work.tile([128, D], FP, tag="wln_t")
            nc.vector.tensor_scalar_mul(wln_t[:], wlts[t][:],
                                        winv[:, ts(t, 1)])
            nc.vector.scalar_tensor_tensor(
                out=dump[:], in0=fn32[t][:], scalar=1.0, in1=wln_t[:],
                op0=OP.mult, op1=OP.mult,
                accum_out=cosl[:, ts(t, 1)],
            )

        # margin math on [128, 8]
        nc.vector.tensor_scalar(cosl[:], cosl[:], -1.0, 1.0, OP.max, OP.min)
        sq = small.tile([128, NT], FP, tag="sq")
        nc.vector.tensor_mul(sq[:], cosl[:], cosl[:])
        sin2 = small.tile([128, NT], FP, tag="sin2")
        nc.vector.tensor_scalar(sin2[:], sq[:], -1.0, 1.0 + 1e-5,
                                OP.mult, OP.add)
        # sin = sin2 * rsqrt(sin2)  (DVE only)
        rs2 = dve_rsqrt(nc, small, sin2[:], 128, NT, "rss", iters=2)
        sinl = small.tile([128, NT], FP, tag="sinl")
        nc.vector.tensor_mul(sinl[:], sin2[:], rs2)
        cosm = small.tile([128, NT], FP, tag="cosm")
        sinm = small.tile([128, NT], FP, tag="sinm")
        nc.vector.tensor_scalar_mul(sinm[:], sinl[:], SIN_M)
        nc.vector.scalar_tensor_tensor(
            out=cosm[:], in0=cosl[:], scalar=COS_M, in1=sinm[:],
            op0=OP.mult, op1=OP.subtract,
        )
        other = small.tile([128, NT], FP, tag="other")
        nc.vector.scalar_tensor_tensor(
            out=other[:], in0=sinl[:], scalar=-MARGIN, in1=cosl[:],
            op0=OP.mult, op1=OP.add,
        )
        mask = small.tile([128, NT], mybir.dt.uint8, tag="mask")
        nc.vector.tensor_single_scalar(mask[:], cosl[:], MIN_COS, OP.is_gt)
        target = small.tile([128, NT], FP, tag="target")
        nc.vector.select(target[:], mask[:], cosm[:], other[:])
        tlog = small.tile([128, NT], FP, tag="tlog")
        nc.vector.tensor_scalar_mul(tlog[:], target[:], S)
        e1 = small.tile([128, NT], FP, tag="e1")
        nc.scalar.activation(e1[:], target[:], AF.Exp, bias=FB, scale=S)
        e2 = small.tile([128, NT], FP, tag="e2")
        nc.scalar.activation(e2[:], cosl[:], AF.Exp, bias=FB, scale=S)
        delta = small.tile([128, NT], FP, tag="delta")
        nc.vector.tensor_sub(delta[:], e1[:], e2[:])

        # ---- all-reduce partial sums, finish loss ----
        nc.sync.dma_start(out=cc_in.ap(), in_=pall[:])
        nc.gpsimd.collective_compute(
            "AllReduce",
            OP.add,
            replica_groups=[list(range(NCORES))],
            ins=[cc_in.ap().opt()],
            outs=[cc_out.ap().opt()],
        )
        pg = msmall.tile([128, NT], FP, tag="pg")
        nc.sync.dma_start(out=pg[:], in_=cc_out.ap())

        u = msmall.tile([128, NT], FP, tag="u")
        nc.vector.tensor_add(u[:], pg[:], delta[:])
        # HW Ln is inaccurate for tiny args; rescale by an exact 2^70
        lnu = msmall.tile([128, NT], FP, tag="lnu")
        nc.scalar.activation(lnu[:], u[:], AF.Ln, bias=0.0, scale=2.0 ** 70)
        nll = msmall.tile([128, NT], FP, tag="nll")
        nc.vector.scalar_tensor_tensor(
            out=nll[:], in0=lnu[:], scalar=(-FB - 70.0 * LN2), in1=tlog[:],
            op0=OP.add, op1=OP.subtract,
        )
        nsum = msmall.tile([128, 1], FP, tag="nsum")
        nc.vector.tensor_reduce(nsum[:], nll[:],
                                axis=mybir.AxisListType.X, op=OP.add)
        ones_fp2 = msmall.tile([128, 1], FP, tag="ones_fp2")
        nc.vector.memset(ones_fp2[:], 1.0)
        lp = psumn2.tile([1, 1], FP, tag="n2p", bufs=1)
        nc.tensor.matmul(lp[:], ones_fp2[:], nsum[:], start=True, stop=True)
        res = msmall.tile([1, 1], FP, tag="res")
        nc.scalar.activation(res[:], lp[:], AF.Copy, bias=0.0, scale=1.0 / N)
        nc.sync.dma_start(out=out_d.ap(), in_=res[:])

    nc.compile()
    return nc


_NC_CACHE = None


def _get_nc():
    global _NC_CACHE
    if _NC_CACHE is None:
        _NC_CACHE = build_nc()
    return _NC_CACHE


def _make_in_maps(feats, w, labels):
    feats = np.asarray(feats, dtype=np.float32).reshape(N, D)
    w = np.asarray(w, dtype=np.float32)
    labels = np.asarray(labels).astype(np.int64)
    wl = np.ascontiguousarray(w[labels]).astype(np.float32)
    in_maps = []
    for i in range(NCORES):
        wt = np.zeros((D, CP), dtype=ml_dtypes.bfloat16)
        wt[:, :CS] = np.ascontiguousarray(
            w[i * CS:(i + 1) * CS].T
        ).astype(ml_dtypes.bfloat16)
        in_maps.append({"wt": wt, "feats": feats, "wl": wl})
    return in_maps


def run(feats, w, labels, trace=False):
    nc = _get_nc()
    in_maps = _make_in_maps(feats, w, labels)
    res = run_bass_kernel_spmd(nc, in_maps, core_ids=list(range(NCORES)),
                               trace=trace)
    out = np.asarray(res.results[0]["out"], dtype=np.float32).reshape(())
    return out, res


def kernel(feats, w, labels):
    out, _ = run(feats, w, labels)
    return out


# revision 36
# speedup vs baseline: 1.3605x; 1.3605x over previous
"""ArcFace loss (margin softmax CE) on 8 TRN2 NeuronCores.

Strategy (model-parallel softmax CE, classes sharded over 8 cores):
  - host: shard W row-wise by class (12500/core, zero-padded to 12544),
    transpose to [512, Cp] and cast bf16; gather wl = w[labels] (layout
    prep only - all math runs on device).
  - device (SPMD, identical graph on all cores), classes-on-partitions:
      * normalize feats; build fnT (d-major) via PE transposes.
      * per-class 1/||w_c|| via ones-matmul over squared wT chunks,
        redistributed to a [128, 98] per-partition grid through a tiny
        DRAM round-trip; rsqrt on the VectorEngine (bit seed + Newton)
        so ScalarE runs ONLY Exp in the hot region (single ACT table).
      * main, per 128-class tile: Z[c,n] = wT.T @ fnT (bf16, PSUM f32);
        ACT exp(S*inv_c * z - 64) with the per-class scale as the
        activation's per-partition scale operand - W itself is never
        normalized, stays read-only. Fixed max 64 = S*max|cos| keeps
        all terms <= 1. Partial softmax sums accumulate on the DVE in
        bf16; one final ones-matmul reduces over class partitions.
      * label-margin path computed redundantly on every core from wl.
      * AllReduce(add) the [1024] partial sums (a tiny early dummy
        collective absorbs the ncfw first-collective setup cost);
        each core finishes loss = mean(64 + ln(P + delta) - S*t),
        with Ln rescaled by an exact 2^70 (HW Ln is inaccurate for
        ~1e-21 arguments).
"""

import math

import numpy as np
import ml_dtypes

import concourse.bass as bass
import concourse.tile as tile
from concourse import bacc, mybir
from concourse.bass import ts, ds
from concourse.bass_utils import run_bass_kernel_spmd
from concourse.masks import make_identity

FP = mybir.dt.float32
BF = mybir.dt.bfloat16
I32 = mybir.dt.int32
AF = mybir.ActivationFunctionType
OP = mybir.AluOpType

# problem constants (hardcoded per harness contract)
MARGIN = 0.5
S = 64.0
COS_M = math.cos(MARGIN)
SIN_M = math.sin(MARGIN)
MIN_COS = math.cos(math.pi - MARGIN)
C = 100000
D = 512
N = 1024
NCORES = 8
CS = C // NCORES          # 12500 classes per core
CP = 12544                # padded: 98 * 128
NT = N // 128             # 8 row tiles
NCT = CP // 128           # 98 class tiles of 128
FB = -64.0                # fixed log-domain shift (= -S * max cos)
LN2 = math.log(2.0)
RSQRT_MAGIC = float(0x5F3759DF)

# W column groups (separate SBUF tiles; small first group = fast ramp)
NGRP = 6
GRPOFF = [0, 1024, 3584, 6144, 8704, 11264]
GRPSZ = [1024, 2560, 2560, 2560, 2560, 1280]


def _subchunks(gsz):
    out, o = [], 0
    while o < gsz:
        s = min(512, gsz - o)
        out.append((o, s))
        o += s
    return out


SUBCH = [_subchunks(s) for s in GRPSZ]


def dve_rsqrt(nc, pool, x_ap, rows, cols, prefix, iters=2):
    """1/sqrt(x) on the VectorEngine only: quake-style bit seed via a
    float round-trip (no int multiply needed), then Newton iterations.
    x > 0, f32. Returns the result AP ([rows, cols] f32)."""
    xs = pool.tile([rows, cols], I32, tag=f"{prefix}_xs", name=f"{prefix}_xs")
    nc.vector.tensor_single_scalar(xs[:], x_ap.bitcast(I32), 1,
                                   OP.logical_shift_right)
    xf = pool.tile([rows, cols], FP, tag=f"{prefix}_xf", name=f"{prefix}_xf")
    nc.vector.tensor_copy(xf[:], xs[:])
    yf = pool.tile([rows, cols], FP, tag=f"{prefix}_yf", name=f"{prefix}_yf")
    nc.vector.tensor_scalar(yf[:], xf[:], -1.0, RSQRT_MAGIC, OP.mult, OP.add)
    yi = pool.tile([rows, cols], I32, tag=f"{prefix}_yi", name=f"{prefix}_yi")
    nc.vector.tensor_copy(yi[:], yf[:])
    y = yi[:].bitcast(FP)
    for it in range(iters):
        t1 = pool.tile([rows, cols], FP, tag=f"{prefix}_t1{it}",
                       name=f"{prefix}_t1_{it}")
        nc.vector.tensor_mul(t1[:], y, y)
        nc.vector.tensor_mul(t1[:], t1[:], x_ap)
        nc.vector.tensor_scalar(t1[:], t1[:], -0.5, 1.5, OP.mult, OP.add)
        yn = pool.tile([rows, cols], FP, tag=f"{prefix}_yn{it}",
                       name=f"{prefix}_yn_{it}")
        nc.vector.tensor_mul(yn[:], y, t1[:])
        y = yn[:]
    return y


def build_nc():
    nc = bacc.Bacc(
        "TRN2",
        target_bir_lowering=False,
        debug=False,
        enable_asserts=False,
        num_devices=NCORES,
    )

    # activation-bias constants must be pre-registered as const APs
    for val in (FB,):
        t = nc.alloc_sbuf_tensor(f"const-f32-{val}", [128, 1], FP)
        nc.gpsimd.memset(t.ap(), val)
        nc.const_aps.aps[(FP, val)] = t.ap()
    nc.all_engine_barrier()

    wt_d = nc.dram_tensor("wt", [D, CP], BF, kind="ExternalInput")
    feats_d = nc.dram_tensor("feats", [N, D], FP, kind="ExternalInput")
    wl_d = nc.dram_tensor("wl", [N, D], FP, kind="ExternalInput")
    out_d = nc.dram_tensor("out", [1, 1], FP, kind="ExternalOutput")

    n2_d = nc.dram_tensor("n2scratch", [1, CP], FP)
    cc_in = nc.dram_tensor("cc_in", [N], FP)
    cc_out = nc.dram_tensor("cc_out", [N], FP)
    ccw_in = nc.dram_tensor("ccw_in", [128, 1], FP)
    ccw_out = nc.dram_tensor("ccw_out", [128, 1], FP)

    # [128, 98] strided view of the n2 scratch (c = ct*128 + p)
    n2_grid = n2_d.ap().rearrange("a (g p) -> (a p) g", p=128)
    # the all-reduced sums come back in (p, t) layout, n = t*128 + p
    ccout_grid = cc_out.ap().rearrange("(t p) -> p t", p=128)

    with tile.TileContext(nc) as tc, (
        tc.tile_pool(name="const", bufs=1)
    ) as constp, (
        tc.tile_pool(name="wres", bufs=1)
    ) as wres, (
        tc.tile_pool(name="fres", bufs=1)
    ) as fres, (
        tc.tile_pool(name="small", bufs=1)
    ) as small, (
        tc.tile_pool(name="work", bufs=3)
    ) as work, (
        tc.tile_pool(name="msmall", bufs=1)
    ) as msmall, (
        tc.tile_pool(name="zpsum", bufs=3, space="PSUM")
    ) as zpsum, (
        tc.tile_pool(name="psumx", bufs=1, space="PSUM")
    ) as psumx:
        identity = constp.tile([128, 128], BF, tag="identity")
        make_identity(nc, identity[:])
        ones_bf = constp.tile([128, 1], BF, tag="ones_bf")
        nc.vector.memset(ones_bf[:], 1.0)

        # small input loads first so they don't queue behind 12.8MB of W
        fts = []
        for t in range(NT):
            f_t = fres.tile([128, D], FP, tag=f"f_{t}", name=f"f_{t}")
            fts.append(f_t)
            nc.sync.dma_start(out=f_t[:], in_=feats_d.ap()[ts(t, 128), :])
        wlts = []
        for t in range(NT):
            wl_t = fres.tile([128, D], FP, tag=f"wl_{t}", name=f"wl_{t}")
            wlts.append(wl_t)
            nc.sync.dma_start(out=wl_t[:], in_=wl_d.ap()[ts(t, 128), :])

        # ---- W load: per (d-chunk, group) tiles, read-only thereafter
        wsb = [[None] * NGRP for _ in range(4)]
        for g in range(NGRP):
            for j in range(4):
                wj = wres.tile([128, GRPSZ[g]], BF, tag=f"wsb{j}_{g}",
                               name=f"wsb{j}_{g}")
                wsb[j][g] = wj
                nc.sync.dma_start(
                    out=wj[:],
                    in_=wt_d.ap()[ts(j, 128), ds(GRPOFF[g], GRPSZ[g])],
                )

        # ---- feats prep: batched row norms, fnT via PE transpose ----
        ssq = small.tile([128, NT], FP, tag="ssq")
        dump = work.tile([128, D], FP, tag="dump", bufs=4)
        for t in range(NT):
            nc.vector.scalar_tensor_tensor(
                out=dump[:], in0=fts[t][:], scalar=1.0, in1=fts[t][:],
                op0=OP.mult, op1=OP.mult,
                accum_out=ssq[:, ts(t, 1)],
            )
        inv_f = dve_rsqrt(nc, small, ssq[:], 128, NT, "rsf", iters=2)

        fn32 = []   # normalized feats, f32, natural layout (label path)
        fnT = [
            fres.tile([128, N], BF, tag=f"fnT{j}", name=f"fnT{j}")
            for j in range(4)
        ]
        for t in range(NT):
            fn_t = fres.tile([128, D], FP, tag=f"fn32_{t}", name=f"fn32_{t}")
            fn32.append(fn_t)
            nc.scalar.mul(fn_t[:], fts[t][:], inv_f[:, ts(t, 1)])
            fnb_t = work.tile([128, D], BF, tag="fnb_t")
            nc.scalar.mul(fnb_t[:], fts[t][:], inv_f[:, ts(t, 1)])
            for j in range(4):
                tp = psumx.tile([128, 128], BF, tag="tp")
                nc.tensor.transpose(tp[:], fnb_t[:, ts(j, 128)], identity[:])
                nc.vector.tensor_copy(fnT[j][:, ts(t, 128)], tp[:])

        # per-class scale S/||w_c|| in [128, NCT] grid layout (c=ct*128+p)
        sinv = small.tile([128, NCT], FP, tag="sinv")
        # bf16 accumulator for the partial softmax sums over class tiles
        acc = msmall.tile([128, N], BF, tag="acc")
        nc.vector.memset(acc[:], 0.0)

        def emit_norm_block(g):
            gall, gsz = GRPOFF[g], GRPSZ[g]
            # n2[c] = sum_d wT[d,c]^2 via ones-matmul over squared chunks
            for c0, csz in SUBCH[g]:
                n2p = psumx.tile([1, 512], FP, tag="n2p", bufs=1,
                                 name=f"n2p_{g}_{c0}")
                for j in range(4):
                    wsq = work.tile([128, 512], BF, tag="wsq",
                                    name=f"wsq_{g}_{c0}_{j}")
                    nc.vector.tensor_mul(wsq[:, :csz],
                                         wsb[j][g][:, ds(c0, csz)],
                                         wsb[j][g][:, ds(c0, csz)])
                    nc.tensor.matmul(
                        n2p[:, :csz], ones_bf[:], wsq[:, :csz],
                        start=(j == 0), stop=(j == 3),
                    )
                n2c = work.tile([1, 512], FP, tag="n2c",
                                name=f"n2c_{g}_{c0}")
                nc.vector.tensor_copy(n2c[:, :csz], n2p[:, :csz])
                nc.gpsimd.dma_start(out=n2_d.ap()[:, ds(gall + c0, csz)],
                                    in_=n2c[:, :csz])

            # sinv = S * rsqrt(n2 + eps) in grid layout (DVE only)
            gcols = gsz // 128
            gw0 = gall // 128
            n2g = work.tile([128, 20], FP, tag="n2g", name=f"n2g_{g}")
            nc.gpsimd.dma_start(out=n2g[:, :gcols],
                                in_=n2_grid[:, ds(gw0, gcols)])
            # guard pad columns (n2 == 0): +1e-24 keeps rsqrt finite
            nc.vector.tensor_scalar_add(n2g[:, :gcols], n2g[:, :gcols], 1e-24)
            invw = dve_rsqrt(nc, work, n2g[:, :gcols], 128, gcols,
                             "rsg", iters=1)
            nc.vector.tensor_scalar_mul(sinv[:, ds(gw0, gcols)], invw, S)

        def emit_main(g):
            gall, gsz = GRPOFF[g], GRPSZ[g]
            for lc in range(gsz // 128):
                ct = gall // 128 + lc
                z = zpsum.tile([128, N], FP, tag="z", name=f"z_{ct}")
                for j in range(4):
                    for h in range(0, N, 512):
                        nc.tensor.matmul(
                            z[:, ds(h, 512)],
                            wsb[j][g][:, ts(lc, 128)],
                            fnT[j][:, ds(h, 512)],
                            start=(j == 0), stop=(j == 3),
                        )
                pd = work.tile([128, N], BF, tag="pd", name=f"pd_{ct}")
                nc.scalar.activation(
                    pd[:], z[:], AF.Exp,
                    bias=FB, scale=sinv[:, ts(ct, 1)],
                )
                nc.vector.tensor_add(acc[:], acc[:], pd[:])

        # pipelined emission: norm chains two groups ahead of the mains
        emit_norm_block(0)
        emit_norm_block(1)
        for g in range(NGRP):
            if g + 2 <= NGRP - 1:
                emit_norm_block(g + 2)
            if g == NGRP - 2:
                # warm up the collective path late in the run: absorbs
                # the ncfw first-collective setup AND re-aligns core
                # skew shortly before the real all-reduce
                warm = constp.tile([128, 1], FP, tag="warm")
                nc.vector.memset(warm[:], 0.0)
                nc.sync.dma_start(out=ccw_in.ap(), in_=warm[:])
                nc.gpsimd.collective_compute(
                    "AllReduce",
                    OP.add,
                    replica_groups=[list(range(NCORES))],
                    ins=[ccw_in.ap().opt()],
                    outs=[ccw_out.ap().opt()],
                )
            emit_main(g)

        # ---- label-margin path (redundant on every core) ----
        cosl = small.tile([128, NT], FP, tag="cosl")
        wsql = small.tile([128, NT], FP, tag="wsql")
        for t in range(NT):
            nc.vector.scalar_tensor_tensor(
                out=dump[:], in0=wlts[t][:], scalar=1.0, in1=wlts[t][:],
                op0=OP.mult, op1=OP.mult,
                accum_out=wsql[:, ts(t, 1)],
            )
        winv = dve_rsqrt(nc, small, wsql[:], 128, NT, "rsw", iters=2)
        for t in range(NT):
            wln_t = work.tile([128, D], FP, tag="wln_t")
            nc.vector.tensor_scalar_mul(wln_t[:], wlts[t][:],
                                        winv[:, ts(t, 1)])
            nc.vector.scalar_tensor_tensor(
                out=dump[:], in0=fn32[t][:], scalar=1.0, in1=wln_t[:],
                op0=OP.mult, op1=OP.mult,
                accum_out=cosl[:, ts(t, 1)],
            )

        # margin math on [128, 8]
        nc.vector.tensor_scalar(cosl[:], cosl[:], -1.0, 1.0, OP.max, OP.min)
        sq = small.tile([128, NT], FP, tag="sq")
        nc.vector.tensor_mul(sq[:], cosl[:], cosl[:])
        sin2 = small.tile([128, NT], FP, tag="sin2")
        nc.vector.tensor_scalar(sin2[:], sq[:], -1.0, 1.0 + 1e-5,
                                OP.mult, OP.add)
        # sin = sin2 * rsqrt(sin2)  (DVE only)
        rs2 = dve_rsqrt(nc, small, sin2[:], 128, NT, "rss", iters=2)
        sinl = small.tile([128, NT], FP, tag="sinl")
        nc.vector.tensor_mul(sinl[:], sin2[:], rs2)
        cosm = small.tile([128, NT], FP, tag="cosm")
        sinm = small.tile([128, NT], FP, tag="sinm")
        nc.vector.tensor_scalar_mul(sinm[:], sinl[:], SIN_M)
        nc.vector.scalar_tensor_tensor(
            out=cosm[:], in0=cosl[:], scalar=COS_M, in1=sinm[:],
            op0=OP.mult, op1=OP.subtract,
        )
        other = small.tile([128, NT], FP, tag="other")
        nc.vector.scalar_tensor_tensor(
            out=other[:], in0=sinl[:], scalar=-MARGIN, in1=cosl[:],
            op0=OP.mult, op1=OP.add,
        )
        mask = small.tile([128, NT], mybir.dt.uint8, tag="mask")
        nc.vector.tensor_single_scalar(mask[:], cosl[:], MIN_COS, OP.is_gt)
        target = small.tile([128, NT], FP, tag="target")
        nc.vector.select(target[:], mask[:], cosm[:], other[:])
        tlog = small.tile([128, NT], FP, tag="tlog")
        nc.vector.tensor_scalar_mul(tlog[:], target[:], S)
        e1 = small.tile([128, NT], FP, tag="e1")
        nc.scalar.activation(e1[:], target[:], AF.Exp, bias=FB, scale=S)
        e2 = small.tile([128, NT], FP, tag="e2")
        nc.scalar.activation(e2[:], cosl[:], AF.Exp, bias=FB, scale=S)
        delta = small.tile([128, NT], FP, tag="delta")
        nc.vector.tensor_sub(delta[:], e1[:], e2[:])

        # ---- reduce over class partitions, all-reduce, finish loss ----
        psum_rows = [
            psumx.tile([1, 512], FP, tag=("tp" if h == 0 else "n2p"),
                       name=f"rowp{h}")
            for h in range(2)
        ]
        for h in range(2):
            nc.tensor.matmul(psum_rows[h][:], ones_bf[:],
                             acc[:, ds(h * 512, 512)], start=True, stop=True)
        prow = msmall.tile([1, N], FP, tag="prow")
        for h in range(2):
            nc.vector.tensor_copy(prow[:, ds(h * 512, 512)],
                                  psum_rows[h][:])
        nc.sync.dma_start(out=cc_in.ap(), in_=prow[:])
        nc.gpsimd.collective_compute(
            "AllReduce",
            OP.add,
            replica_groups=[list(range(NCORES))],
            ins=[cc_in.ap().opt()],
            outs=[cc_out.ap().opt()],
        )
        pg = msmall.tile([128, NT], FP, tag="pg")
        nc.gpsimd.dma_start(out=pg[:], in_=ccout_grid)

        u = msmall.tile([128, NT], FP, tag="u")
        nc.vector.tensor_add(u[:], pg[:], delta[:])
        # HW Ln is inaccurate for tiny args; rescale by an exact 2^70
        lnu = msmall.tile([128, NT], FP, tag="lnu")
        nc.scalar.activation(lnu[:], u[:], AF.Ln, bias=0.0, scale=2.0 ** 70)
        nll = msmall.tile([128, NT], FP, tag="nll")
        nc.vector.scalar_tensor_tensor(
            out=nll[:], in0=lnu[:], scalar=(-FB - 70.0 * LN2), in1=tlog[:],
            op0=OP.add, op1=OP.subtract,
        )
        nsum = msmall.tile([128, 1], FP, tag="nsum")
        nc.vector.tensor_reduce(nsum[:], nll[:],
                                axis=mybir.AxisListType.X, op=OP.add)
        ones_fp2 = msmall.tile([128, 1], FP, tag="ones_fp2")
        nc.vector.memset(ones_fp2[:], 1.0)
        lp = psumx.tile([1, 1], FP, tag="tp")
        nc.tensor.matmul(lp[:], ones_fp2[:], nsum[:], start=True, stop=True)
        res = msmall.tile([1, 1], FP, tag="res")
        nc.scalar.activation(res[:], lp[:], AF.Copy, bias=0.0, scale=1.0 / N)
        nc.sync.dma_start(out=out_d.ap(), in_=res[:])

    nc.compile()
    return nc


_NC_CACHE = None


def _get_nc():
    global _NC_CACHE
    if _NC_CACHE is None:
        _NC_CACHE = build_nc()
    return _NC_CACHE


def _make_in_maps(feats, w, labels):
    feats = np.asarray(feats, dtype=np.float32).reshape(N, D)
    w = np.asarray(w, dtype=np.float32)
    labels = np.asarray(labels).astype(np.int64)
    wl = np.ascontiguousarray(w[labels]).astype(np.float32)
    in_maps = []
    for i in range(NCORES):
        wt = np.zeros((D, CP), dtype=ml_dtypes.bfloat16)
        wt[:, :CS] = np.ascontiguousarray(
            w[i * CS:(i + 1) * CS].T
        ).astype(ml_dtypes.bfloat16)
        in_maps.append({"wt": wt, "feats": feats, "wl": wl})
    return in_maps


def run(feats, w, labels, trace=False):
    nc = _get_nc()
    in_maps = _make_in_maps(feats, w, labels)
    res = run_bass_kernel_spmd(nc, in_maps, core_ids=list(range(NCORES)),
                               trace=trace)
    out = np.asarray(res.results[0]["out"], dtype=np.float32).reshape(())
    return out, res


def kernel(feats, w, labels):
    out, _ = run(feats, w, labels)
    return out


# revision 38
# speedup vs baseline: 1.3802x; 1.0145x over previous
"""ArcFace loss (margin softmax CE) on 8 TRN2 NeuronCores.

Strategy (model-parallel softmax CE, classes sharded over 8 cores):
  - host: shard W row-wise by class (12500/core, zero-padded to 12544),
    transpose to [512, Cp] and cast bf16; gather wl = w[labels] (layout
    prep only - all math runs on device).
  - device (SPMD, identical graph on all cores), classes-on-partitions:
      * normalize feats; build fnT (d-major) via PE transposes.
      * per-class 1/||w_c|| via ones-matmul over squared wT chunks,
        redistributed to a [128, 98] per-partition grid through a tiny
        DRAM round-trip; rsqrt on the VectorEngine (bit seed + Newton)
        so ScalarE runs ONLY Exp in the hot region (single ACT table).
      * main, per 128-class tile: Z[c,n] = wT.T @ fnT (bf16, PSUM f32);
        ACT exp(S*inv_c * z - 64) with the per-class scale as the
        activation's per-partition scale operand - W itself is never
        normalized, stays read-only. Fixed max 64 = S*max|cos| keeps
        all terms <= 1. Partial softmax sums accumulate on the DVE in
        bf16; one final ones-matmul reduces over class partitions.
      * label-margin path computed redundantly on every core from wl.
      * AllReduce(add) the [1024] partial sums (a tiny early dummy
        collective absorbs the ncfw first-collective setup cost);
        each core finishes loss = mean(64 + ln(P + delta) - S*t),
        with Ln rescaled by an exact 2^70 (HW Ln is inaccurate for
        ~1e-21 arguments).
"""

import math

import numpy as np
import ml_dtypes

import concourse.bass as bass
import concourse.tile as tile
from concourse import bacc, mybir
from concourse.bass import ts, ds
from concourse.bass_utils import run_bass_kernel_spmd
from concourse.masks import make_identity

FP = mybir.dt.float32
BF = mybir.dt.bfloat16
I32 = mybir.dt.int32
AF = mybir.ActivationFunctionType
OP = mybir.AluOpType

# problem constants (hardcoded per harness contract)
MARGIN = 0.5
S = 64.0
COS_M = math.cos(MARGIN)
SIN_M = math.sin(MARGIN)
MIN_COS = math.cos(math.pi - MARGIN)
C = 100000
D = 512
N = 1024
NCORES = 8
CS = C // NCORES          # 12500 classes per core
CP = 12544                # padded: 98 * 128
NT = N // 128             # 8 row tiles
NCT = CP // 128           # 98 class tiles of 128
FB = -64.0                # fixed log-domain shift (= -S * max cos)
LN2 = math.log(2.0)
RSQRT_MAGIC = float(0x5F3759DF)

# W column groups (separate SBUF tiles; small first group = fast ramp)
NGRP = 6
GRPOFF = [0, 1024, 3584, 6144, 8704, 11264]
GRPSZ = [1024, 2560, 2560, 2560, 2560, 1280]


def _subchunks(gsz):
    out, o = [], 0
    while o < gsz:
        s = min(512, gsz - o)
        out.append((o, s))
        o += s
    return out


SUBCH = [_subchunks(s) for s in GRPSZ]


def dve_rsqrt(nc, pool, x_ap, rows, cols, prefix, iters=2):
    """1/sqrt(x) on the VectorEngine only: quake-style bit seed via a
    float round-trip (no int multiply needed), then Newton iterations.
    x > 0, f32. Returns the result AP ([rows, cols] f32)."""
    xs = pool.tile([rows, cols], I32, tag=f"{prefix}_xs", name=f"{prefix}_xs")
    nc.vector.tensor_single_scalar(xs[:], x_ap.bitcast(I32), 1,
                                   OP.logical_shift_right)
    xf = pool.tile([rows, cols], FP, tag=f"{prefix}_xf", name=f"{prefix}_xf")
    nc.vector.tensor_copy(xf[:], xs[:])
    yf = pool.tile([rows, cols], FP, tag=f"{prefix}_yf", name=f"{prefix}_yf")
    nc.vector.tensor_scalar(yf[:], xf[:], -1.0, RSQRT_MAGIC, OP.mult, OP.add)
    yi = pool.tile([rows, cols], I32, tag=f"{prefix}_yi", name=f"{prefix}_yi")
    nc.vector.tensor_copy(yi[:], yf[:])
    y = yi[:].bitcast(FP)
    for it in range(iters):
        t1 = pool.tile([rows, cols], FP, tag=f"{prefix}_t1{it}",
                       name=f"{prefix}_t1_{it}")
        nc.vector.tensor_mul(t1[:], y, y)
        nc.vector.tensor_mul(t1[:], t1[:], x_ap)
        nc.vector.tensor_scalar(t1[:], t1[:], -0.5, 1.5, OP.mult, OP.add)
        yn = pool.tile([rows, cols], FP, tag=f"{prefix}_yn{it}",
                       name=f"{prefix}_yn_{it}")
        nc.vector.tensor_mul(yn[:], y, t1[:])
        y = yn[:]
    return y


def build_nc():
    nc = bacc.Bacc(
        "TRN2",
        target_bir_lowering=False,
        debug=False,
        enable_asserts=False,
        num_devices=NCORES,
    )

    # activation-bias constants must be pre-registered as const APs
    for val in (FB,):
        t = nc.alloc_sbuf_tensor(f"const-f32-{val}", [128, 1], FP)
        nc.gpsimd.memset(t.ap(), val)
        nc.const_aps.aps[(FP, val)] = t.ap()
    nc.all_engine_barrier()

    wt_d = nc.dram_tensor("wt", [D, CP], BF, kind="ExternalInput")
    feats_d = nc.dram_tensor("feats", [N, D], FP, kind="ExternalInput")
    wl_d = nc.dram_tensor("wl", [N, D], FP, kind="ExternalInput")
    out_d = nc.dram_tensor("out", [1, 1], FP, kind="ExternalOutput")

    n2_d = nc.dram_tensor("n2scratch", [1, CP], FP)
    cc_in = nc.dram_tensor("cc_in", [N], FP)
    cc_out = nc.dram_tensor("cc_out", [N], FP)
    ccw_in = nc.dram_tensor("ccw_in", [128, 1], FP)
    ccw_out = nc.dram_tensor("ccw_out", [128, 1], FP)

    # [128, 98] strided view of the n2 scratch (c = ct*128 + p)
    n2_grid = n2_d.ap().rearrange("a (g p) -> (a p) g", p=128)
    # the all-reduced sums come back in (p, t) layout, n = t*128 + p
    ccout_grid = cc_out.ap().rearrange("(t p) -> p t", p=128)

    with tile.TileContext(nc) as tc, (
        tc.tile_pool(name="const", bufs=1)
    ) as constp, (
        tc.tile_pool(name="wres", bufs=1)
    ) as wres, (
        tc.tile_pool(name="fres", bufs=1)
    ) as fres, (
        tc.tile_pool(name="small", bufs=1)
    ) as small, (
        tc.tile_pool(name="work", bufs=3)
    ) as work, (
        tc.tile_pool(name="msmall", bufs=1)
    ) as msmall, (
        tc.tile_pool(name="zpsum", bufs=3, space="PSUM")
    ) as zpsum, (
        tc.tile_pool(name="psumx", bufs=1, space="PSUM")
    ) as psumx:
        identity = constp.tile([128, 128], BF, tag="identity")
        make_identity(nc, identity[:])
        ones_bf = constp.tile([128, 1], BF, tag="ones_bf")
        nc.vector.memset(ones_bf[:], 1.0)

        # W group 0 first (feeds the first norm chain), then the small
        # inputs, then the rest of W
        wsb = [[None] * NGRP for _ in range(4)]
        for g in (0,):
            for j in range(4):
                wj = wres.tile([128, GRPSZ[g]], BF, tag=f"wsb{j}_{g}",
                               name=f"wsb{j}_{g}")
                wsb[j][g] = wj
                nc.sync.dma_start(
                    out=wj[:],
                    in_=wt_d.ap()[ts(j, 128), ds(GRPOFF[g], GRPSZ[g])],
                )
        fts = []
        for t in range(NT):
            f_t = fres.tile([128, D], FP, tag=f"f_{t}", name=f"f_{t}")
            fts.append(f_t)
            nc.sync.dma_start(out=f_t[:], in_=feats_d.ap()[ts(t, 128), :])
        wlts = []
        for t in range(NT):
            wl_t = fres.tile([128, D], FP, tag=f"wl_{t}", name=f"wl_{t}")
            wlts.append(wl_t)
            nc.sync.dma_start(out=wl_t[:], in_=wl_d.ap()[ts(t, 128), :])

        # ---- W load: per (d-chunk, group) tiles, read-only thereafter
        for g in range(1, NGRP):
            for j in range(4):
                wj = wres.tile([128, GRPSZ[g]], BF, tag=f"wsb{j}_{g}",
                               name=f"wsb{j}_{g}")
                wsb[j][g] = wj
                nc.sync.dma_start(
                    out=wj[:],
                    in_=wt_d.ap()[ts(j, 128), ds(GRPOFF[g], GRPSZ[g])],
                )

        # ---- feats prep: batched row norms, fnT via PE transpose ----
        ssq = small.tile([128, NT], FP, tag="ssq")
        dump = work.tile([128, D], FP, tag="dump", bufs=4)
        for t in range(NT):
            nc.vector.scalar_tensor_tensor(
                out=dump[:], in0=fts[t][:], scalar=1.0, in1=fts[t][:],
                op0=OP.mult, op1=OP.mult,
                accum_out=ssq[:, ts(t, 1)],
            )
        inv_f = dve_rsqrt(nc, small, ssq[:], 128, NT, "rsf", iters=2)

        fn32 = []   # normalized feats, f32, natural layout (label path)
        fnT = [
            fres.tile([128, N], BF, tag=f"fnT{j}", name=f"fnT{j}")
            for j in range(4)
        ]
        for t in range(NT):
            fn_t = fres.tile([128, D], FP, tag=f"fn32_{t}", name=f"fn32_{t}")
            fn32.append(fn_t)
            nc.scalar.mul(fn_t[:], fts[t][:], inv_f[:, ts(t, 1)])
            fnb_t = work.tile([128, D], BF, tag="fnb_t")
            nc.scalar.mul(fnb_t[:], fts[t][:], inv_f[:, ts(t, 1)])
            for j in range(4):
                tp = psumx.tile([128, 128], BF, tag="tp")
                nc.tensor.transpose(tp[:], fnb_t[:, ts(j, 128)], identity[:])
                nc.vector.tensor_copy(fnT[j][:, ts(t, 128)], tp[:])

        # per-class scale S/||w_c|| in [128, NCT] grid layout (c=ct*128+p)
        sinv = small.tile([128, NCT], FP, tag="sinv")
        # bf16 accumulator for the partial softmax sums over class tiles
        acc = msmall.tile([128, N], BF, tag="acc")
        nc.vector.memset(acc[:], 0.0)

        def emit_norm_block(g):
            gall, gsz = GRPOFF[g], GRPSZ[g]
            # n2[c] = sum_d wT[d,c]^2 via ones-matmul over squared chunks
            for c0, csz in SUBCH[g]:
                n2p = psumx.tile([1, 512], FP, tag="n2p", bufs=1,
                                 name=f"n2p_{g}_{c0}")
                for j in range(4):
                    wsq = work.tile([128, 512], BF, tag="wsq",
                                    name=f"wsq_{g}_{c0}_{j}")
                    nc.vector.tensor_mul(wsq[:, :csz],
                                         wsb[j][g][:, ds(c0, csz)],
                                         wsb[j][g][:, ds(c0, csz)])
                    nc.tensor.matmul(
                        n2p[:, :csz], ones_bf[:], wsq[:, :csz],
                        start=(j == 0), stop=(j == 3),
                    )
                n2c = work.tile([1, 512], FP, tag="n2c",
                                name=f"n2c_{g}_{c0}")
                nc.vector.tensor_copy(n2c[:, :csz], n2p[:, :csz])
                nc.gpsimd.dma_start(out=n2_d.ap()[:, ds(gall + c0, csz)],
                                    in_=n2c[:, :csz])

            # sinv = S * rsqrt(n2 + eps) in grid layout (DVE only)
            gcols = gsz // 128
            gw0 = gall // 128
            n2g = work.tile([128, 20], FP, tag="n2g", name=f"n2g_{g}")
            nc.gpsimd.dma_start(out=n2g[:, :gcols],
                                in_=n2_grid[:, ds(gw0, gcols)])
            # guard pad columns (n2 == 0): +1e-24 keeps rsqrt finite
            nc.vector.tensor_scalar_add(n2g[:, :gcols], n2g[:, :gcols], 1e-24)
            invw = dve_rsqrt(nc, work, n2g[:, :gcols], 128, gcols,
                             "rsg", iters=1)
            nc.vector.tensor_scalar_mul(sinv[:, ds(gw0, gcols)], invw, S)

        def emit_main(g):
            gall, gsz = GRPOFF[g], GRPSZ[g]
            for lc in range(gsz // 128):
                ct = gall // 128 + lc
                z = zpsum.tile([128, N], FP, tag="z", name=f"z_{ct}")
                for j in range(4):
                    for h in range(0, N, 512):
                        nc.tensor.matmul(
                            z[:, ds(h, 512)],
                            wsb[j][g][:, ts(lc, 128)],
                            fnT[j][:, ds(h, 512)],
                            start=(j == 0), stop=(j == 3),
                        )
                pd = work.tile([128, N], BF, tag="pd", name=f"pd_{ct}")
                nc.scalar.activation(
                    pd[:], z[:], AF.Exp,
                    bias=FB, scale=sinv[:, ts(ct, 1)],
                )
                nc.vector.tensor_add(acc[:], acc[:], pd[:])

        def emit_label_path():
            # ---- label-margin path (redundant on every core) ----
            cosl = small.tile([128, NT], FP, tag="cosl")
            wsql = small.tile([128, NT], FP, tag="wsql")
            for t in range(NT):
                nc.vector.scalar_tensor_tensor(
                    out=dump[:], in0=wlts[t][:], scalar=1.0, in1=wlts[t][:],
                    op0=OP.mult, op1=OP.mult,
                    accum_out=wsql[:, ts(t, 1)],
                )
            winv = dve_rsqrt(nc, small, wsql[:], 128, NT, "rsw", iters=2)
            for t in range(NT):
                wln_t = work.tile([128, D], FP, tag="wln_t")
                nc.vector.tensor_scalar_mul(wln_t[:], wlts[t][:],
                                            winv[:, ts(t, 1)])
                nc.vector.scalar_tensor_tensor(
                    out=dump[:], in0=fn32[t][:], scalar=1.0, in1=wln_t[:],
                    op0=OP.mult, op1=OP.mult,
                    accum_out=cosl[:, ts(t, 1)],
                )

            # margin math on [128, 8]
            nc.vector.tensor_scalar(cosl[:], cosl[:], -1.0, 1.0, OP.max, OP.min)
            sq = small.tile([128, NT], FP, tag="sq")
            nc.vector.tensor_mul(sq[:], cosl[:], cosl[:])
            sin2 = small.tile([128, NT], FP, tag="sin2")
            nc.vector.tensor_scalar(sin2[:], sq[:], -1.0, 1.0 + 1e-5,
                                    OP.mult, OP.add)
            # sin = sin2 * rsqrt(sin2)  (DVE only)
            rs2 = dve_rsqrt(nc, small, sin2[:], 128, NT, "rss", iters=2)
            sinl = small.tile([128, NT], FP, tag="sinl")
            nc.vector.tensor_mul(sinl[:], sin2[:], rs2)
            cosm = small.tile([128, NT], FP, tag="cosm")
            sinm = small.tile([128, NT], FP, tag="sinm")
            nc.vector.tensor_scalar_mul(sinm[:], sinl[:], SIN_M)
            nc.vector.scalar_tensor_tensor(
                out=cosm[:], in0=cosl[:], scalar=COS_M, in1=sinm[:],
                op0=OP.mult, op1=OP.subtract,
            )
            other = small.tile([128, NT], FP, tag="other")
            nc.vector.scalar_tensor_tensor(
                out=other[:], in0=sinl[:], scalar=-MARGIN, in1=cosl[:],
                op0=OP.mult, op1=OP.add,
            )
            mask = small.tile([128, NT], mybir.dt.uint8, tag="mask")
            nc.vector.tensor_single_scalar(mask[:], cosl[:], MIN_COS, OP.is_gt)
            target = small.tile([128, NT], FP, tag="target")
            nc.vector.select(target[:], mask[:], cosm[:], other[:])
            tlog = small.tile([128, NT], FP, tag="tlog")
            nc.vector.tensor_scalar_mul(tlog[:], target[:], S)
            e1 = small.tile([128, NT], FP, tag="e1")
            nc.scalar.activation(e1[:], target[:], AF.Exp, bias=FB, scale=S)
            e2 = small.tile([128, NT], FP, tag="e2")
            nc.scalar.activation(e2[:], cosl[:], AF.Exp, bias=FB, scale=S)
            delta = small.tile([128, NT], FP, tag="delta")
            nc.vector.tensor_sub(delta[:], e1[:], e2[:])

            return tlog, delta


        # pipelined emission: norm chains two groups ahead of the mains
        emit_norm_block(0)
        emit_norm_block(1)
        for g in range(NGRP):
            if g + 2 <= NGRP - 1:
                emit_norm_block(g + 2)
            if g == NGRP - 2:
                # warm up the collective path late in the run: absorbs
                # the ncfw first-collective setup AND re-aligns core
                # skew shortly before the real all-reduce
                warm = constp.tile([128, 1], FP, tag="warm")
                nc.vector.memset(warm[:], 0.0)
                nc.sync.dma_start(out=ccw_in.ap(), in_=warm[:])
                nc.gpsimd.collective_compute(
                    "AllReduce",
                    OP.add,
                    replica_groups=[list(range(NCORES))],
                    ins=[ccw_in.ap().opt()],
                    outs=[ccw_out.ap().opt()],
                )
            emit_main(g)
            if g == 0:
                tlog, delta = emit_label_path()

        # ---- reduce over class partitions, all-reduce, finish loss ----
        psum_rows = [
            psumx.tile([1, 512], FP, tag=("tp" if h == 0 else "n2p"),
                       name=f"rowp{h}")
            for h in range(2)
        ]
        for h in range(2):
            nc.tensor.matmul(psum_rows[h][:], ones_bf[:],
                             acc[:, ds(h * 512, 512)], start=True, stop=True)
        prow = msmall.tile([1, N], FP, tag="prow")
        for h in range(2):
            nc.vector.tensor_copy(prow[:, ds(h * 512, 512)],
                                  psum_rows[h][:])
        nc.sync.dma_start(out=cc_in.ap(), in_=prow[:])
        nc.gpsimd.collective_compute(
            "AllReduce",
            OP.add,
            replica_groups=[list(range(NCORES))],
            ins=[cc_in.ap().opt()],
            outs=[cc_out.ap().opt()],
        )
        pg = msmall.tile([128, NT], FP, tag="pg")
        nc.gpsimd.dma_start(out=pg[:], in_=ccout_grid)

        u = msmall.tile([128, NT], FP, tag="u")
        nc.vector.tensor_add(u[:], pg[:], delta[:])
        # HW Ln is inaccurate for tiny args; rescale by an exact 2^70
        lnu = msmall.tile([128, NT], FP, tag="lnu")
        nc.scalar.activation(lnu[:], u[:], AF.Ln, bias=0.0, scale=2.0 ** 70)
        nll = msmall.tile([128, NT], FP, tag="nll")
        nc.vector.scalar_tensor_tensor(
            out=nll[:], in0=lnu[:], scalar=(-FB - 70.0 * LN2), in1=tlog[:],
            op0=OP.add, op1=OP.subtract,
        )
        nsum = msmall.tile([128, 1], FP, tag="nsum")
        nc.vector.tensor_reduce(nsum[:], nll[:],
                                axis=mybir.AxisListType.X, op=OP.add)
        ones_fp2 = msmall.tile([128, 1], FP, tag="ones_fp2")
        nc.vector.memset(ones_fp2[:], 1.0)
        lp = psumx.tile([1, 1], FP, tag="tp")
        nc.tensor.matmul(lp[:], ones_fp2[:], nsum[:], start=True, stop=True)
        res = msmall.tile([1, 1], FP, tag="res")
        nc.scalar.activation(res[:], lp[:], AF.Copy, bias=0.0, scale=1.0 / N)
        nc.sync.dma_start(out=out_d.ap(), in_=res[:])

    nc.compile()
    return nc


_NC_CACHE = None


def _get_nc():
    global _NC_CACHE
    if _NC_CACHE is None:
        _NC_CACHE = build_nc()
    return _NC_CACHE


def _make_in_maps(feats, w, labels):
    feats = np.asarray(feats, dtype=np.float32).reshape(N, D)
    w = np.asarray(w, dtype=np.float32)
    labels = np.asarray(labels).astype(np.int64)
    wl = np.ascontiguousarray(w[labels]).astype(np.float32)
    in_maps = []
    for i in range(NCORES):
        wt = np.zeros((D, CP), dtype=ml_dtypes.bfloat16)
        wt[:, :CS] = np.ascontiguousarray(
            w[i * CS:(i + 1) * CS].T
        ).astype(ml_dtypes.bfloat16)
        in_maps.append({"wt": wt, "feats": feats, "wl": wl})
    return in_maps


def run(feats, w, labels, trace=False):
    nc = _get_nc()
    in_maps = _make_in_maps(feats, w, labels)
    res = run_bass_kernel_spmd(nc, in_maps, core_ids=list(range(NCORES)),
                               trace=trace)
    out = np.asarray(res.results[0]["out"], dtype=np.float32).reshape(())
    return out, res


def kernel(feats, w, labels):
    out, _ = run(feats, w, labels)
    return out


# revision 40
# speedup vs baseline: 1.7783x; 1.2885x over previous
"""ArcFace loss (margin softmax CE) on 8 TRN2 NeuronCores.

Strategy (model-parallel softmax CE, classes sharded over 8 cores):
  - host: shard W row-wise by class (12500/core, zero-padded to 12544),
    transpose to [512, Cp] and cast bf16; gather wl = w[labels] (layout
    prep only - all math runs on device).
  - device (SPMD, identical graph on all cores), classes-on-partitions:
      * normalize feats; build fnT (d-major) via PE transposes.
      * per-class 1/||w_c|| via ones-matmul over squared wT chunks,
        redistributed to a [128, 98] per-partition grid through a tiny
        DRAM round-trip; rsqrt on the VectorEngine (bit seed + Newton)
        so ScalarE runs ONLY Exp in the hot region (single ACT table).
      * main, per 128-class tile: Z[c,n] = wT.T @ fnT (bf16, PSUM f32);
        ACT exp(S*inv_c * z - 64) with the per-class scale as the
        activation's per-partition scale operand - W itself is never
        normalized, stays read-only. Fixed max 64 = S*max|cos| keeps
        all terms <= 1. Partial softmax sums accumulate on the DVE in
        bf16; one final ones-matmul reduces over class partitions.
      * label-margin path computed redundantly on every core from wl.
      * AllReduce(add) the [1024] partial sums (a tiny early dummy
        collective absorbs the ncfw first-collective setup cost);
        each core finishes loss = mean(64 + ln(P + delta) - S*t),
        with Ln rescaled by an exact 2^70 (HW Ln is inaccurate for
        ~1e-21 arguments).
"""

import math

import numpy as np
import ml_dtypes

import concourse.bass as bass
import concourse.tile as tile
from concourse import bacc, mybir
from concourse.bass import ts, ds
from concourse.bass_utils import run_bass_kernel_spmd
from concourse.masks import make_identity

FP = mybir.dt.float32
BF = mybir.dt.bfloat16
I32 = mybir.dt.int32
F8 = mybir.dt.float8e4
AF = mybir.ActivationFunctionType
OP = mybir.AluOpType

# problem constants (hardcoded per harness contract)
MARGIN = 0.5
S = 64.0
COS_M = math.cos(MARGIN)
SIN_M = math.sin(MARGIN)
MIN_COS = math.cos(math.pi - MARGIN)
C = 100000
D = 512
N = 1024
NCORES = 8
CS = C // NCORES          # 12500 classes per core
CP = 12544                # padded: 98 * 128
NT = N // 128             # 8 row tiles
NCT = CP // 128           # 98 class tiles of 128
FB = -64.0                # fixed log-domain shift (= -S * max cos)
LN2 = math.log(2.0)
RSQRT_MAGIC = float(0x5F3759DF)

# W column groups (separate SBUF tiles; small first group = fast ramp)
NGRP = 6
GRPOFF = [0, 1024, 3584, 6144, 8704, 11264]
GRPSZ = [1024, 2560, 2560, 2560, 2560, 1280]


def _subchunks(gsz):
    out, o = [], 0
    while o < gsz:
        s = min(512, gsz - o)
        out.append((o, s))
        o += s
    return out


SUBCH = [_subchunks(s) for s in GRPSZ]


def dve_rsqrt(nc, pool, x_ap, rows, cols, prefix, iters=2):
    """1/sqrt(x) on the VectorEngine only: quake-style bit seed via a
    float round-trip (no int multiply needed), then Newton iterations.
    x > 0, f32. Returns the result AP ([rows, cols] f32)."""
    xs = pool.tile([rows, cols], I32, tag=f"{prefix}_xs", name=f"{prefix}_xs")
    nc.vector.tensor_single_scalar(xs[:], x_ap.bitcast(I32), 1,
                                   OP.logical_shift_right)
    xf = pool.tile([rows, cols], FP, tag=f"{prefix}_xf", name=f"{prefix}_xf")
    nc.vector.tensor_copy(xf[:], xs[:])
    yf = pool.tile([rows, cols], FP, tag=f"{prefix}_yf", name=f"{prefix}_yf")
    nc.vector.tensor_scalar(yf[:], xf[:], -1.0, RSQRT_MAGIC, OP.mult, OP.add)
    yi = pool.tile([rows, cols], I32, tag=f"{prefix}_yi", name=f"{prefix}_yi")
    nc.vector.tensor_copy(yi[:], yf[:])
    y = yi[:].bitcast(FP)
    for it in range(iters):
        t1 = pool.tile([rows, cols], FP, tag=f"{prefix}_t1{it}",
                       name=f"{prefix}_t1_{it}")
        nc.vector.tensor_mul(t1[:], y, y)
        nc.vector.tensor_mul(t1[:], t1[:], x_ap)
        nc.vector.tensor_scalar(t1[:], t1[:], -0.5, 1.5, OP.mult, OP.add)
        yn = pool.tile([rows, cols], FP, tag=f"{prefix}_yn{it}",
                       name=f"{prefix}_yn_{it}")
        nc.vector.tensor_mul(yn[:], y, t1[:])
        y = yn[:]
    return y


def build_nc():
    nc = bacc.Bacc(
        "TRN2",
        target_bir_lowering=False,
        debug=False,
        enable_asserts=False,
        num_devices=NCORES,
    )

    # activation-bias constants must be pre-registered as const APs
    for val in (FB,):
        t = nc.alloc_sbuf_tensor(f"const-f32-{val}", [128, 1], FP)
        nc.gpsimd.memset(t.ap(), val)
        nc.const_aps.aps[(FP, val)] = t.ap()
    nc.all_engine_barrier()

    wt_d = nc.dram_tensor("wt", [2, 128, 2, CP], F8, kind="ExternalInput")
    feats_d = nc.dram_tensor("feats", [N, D], FP, kind="ExternalInput")
    wl_d = nc.dram_tensor("wl", [N, D], FP, kind="ExternalInput")
    out_d = nc.dram_tensor("out", [1, 1], FP, kind="ExternalOutput")

    n2_d = nc.dram_tensor("n2scratch", [1, CP], FP)
    cc_in = nc.dram_tensor("cc_in", [N], FP)
    cc_out = nc.dram_tensor("cc_out", [N], FP)
    ccw_in = nc.dram_tensor("ccw_in", [128, 1], FP)
    ccw_out = nc.dram_tensor("ccw_out", [128, 1], FP)

    # [128, 98] strided view of the n2 scratch (c = ct*128 + p)
    n2_grid = n2_d.ap().rearrange("a (g p) -> (a p) g", p=128)
    # the all-reduced sums come back in (p, t) layout, n = t*128 + p
    ccout_grid = cc_out.ap().rearrange("(t p) -> p t", p=128)

    with tile.TileContext(nc) as tc, (
        tc.tile_pool(name="const", bufs=1)
    ) as constp, (
        tc.tile_pool(name="wres", bufs=1)
    ) as wres, (
        tc.tile_pool(name="fres", bufs=1)
    ) as fres, (
        tc.tile_pool(name="small", bufs=1)
    ) as small, (
        tc.tile_pool(name="work", bufs=3)
    ) as work, (
        tc.tile_pool(name="msmall", bufs=1)
    ) as msmall, (
        tc.tile_pool(name="zpsum", bufs=3, space="PSUM")
    ) as zpsum, (
        tc.tile_pool(name="psumx", bufs=1, space="PSUM")
    ) as psumx:
        identity = constp.tile([128, 128], BF, tag="identity")
        make_identity(nc, identity[:])
        ones_bf = constp.tile([128, 1], BF, tag="ones_bf")
        nc.vector.memset(ones_bf[:], 1.0)

        # W group 0 first (feeds the first norm chain), then the small
        # inputs, then the rest of W
        wsb = [[None] * NGRP for _ in range(2)]

        def load_w_group(g):
            for jh in range(2):
                wj = wres.tile([128, 2, GRPSZ[g]], F8, tag=f"wsb{jh}_{g}",
                               name=f"wsb{jh}_{g}")
                wsb[jh][g] = wj
                nc.sync.dma_start(
                    out=wj[:],
                    in_=wt_d.ap()[jh, :, :, ds(GRPOFF[g], GRPSZ[g])],
                )

        load_w_group(0)
        fts = []
        for t in range(NT):
            f_t = fres.tile([128, D], FP, tag=f"f_{t}", name=f"f_{t}")
            fts.append(f_t)
            nc.sync.dma_start(out=f_t[:], in_=feats_d.ap()[ts(t, 128), :])
        wlts = []
        for t in range(NT):
            wl_t = fres.tile([128, D], FP, tag=f"wl_{t}", name=f"wl_{t}")
            wlts.append(wl_t)
            nc.sync.dma_start(out=wl_t[:], in_=wl_d.ap()[ts(t, 128), :])

        # ---- W load: per (d-half, group) tiles, read-only thereafter
        for g in range(1, NGRP):
            load_w_group(g)

        # ---- feats prep: batched row norms, fnT via PE transpose ----
        ssq = small.tile([128, NT], FP, tag="ssq")
        dump = work.tile([128, D], FP, tag="dump", bufs=4)
        for t in range(NT):
            nc.vector.scalar_tensor_tensor(
                out=dump[:], in0=fts[t][:], scalar=1.0, in1=fts[t][:],
                op0=OP.mult, op1=OP.mult,
                accum_out=ssq[:, ts(t, 1)],
            )
        inv_f = dve_rsqrt(nc, small, ssq[:], 128, NT, "rsf", iters=2)

        fn32 = []   # normalized feats, f32, natural layout (label path)
        fnT = [
            fres.tile([128, N], BF, tag=f"fnT{j}", name=f"fnT{j}")
            for j in range(4)
        ]
        for t in range(NT):
            fn_t = fres.tile([128, D], FP, tag=f"fn32_{t}", name=f"fn32_{t}")
            fn32.append(fn_t)
            nc.scalar.mul(fn_t[:], fts[t][:], inv_f[:, ts(t, 1)])
            fnb_t = work.tile([128, D], BF, tag="fnb_t")
            nc.scalar.mul(fnb_t[:], fts[t][:], inv_f[:, ts(t, 1)])
            for j in range(4):
                tp = psumx.tile([128, 128], BF, tag="tp")
                nc.tensor.transpose(tp[:], fnb_t[:, ts(j, 128)], identity[:])
                nc.vector.tensor_copy(fnT[j][:, ts(t, 128)], tp[:])

        # fp8 moving operand: cast fnT x16 to fp8, pack d-pairs per
        # partition for DoubleRow ([128, 2, N], d = jh*256 + 2p + i)
        fnT8 = [
            fres.tile([128, N], F8, tag=f"fnT8{b}", name=f"fnT8{b}")
            for b in range(4)
        ]
        for b in range(4):
            nc.vector.tensor_scalar_mul(fnT8[b][:], fnT[b][:], 16.0)
        fnP = [
            fres.tile([128, 2, N], F8, tag=f"fnP{jh}", name=f"fnP{jh}")
            for jh in range(2)
        ]
        for jh in range(2):
            for i in range(2):
                nc.gpsimd.dma_start(
                    out=fnP[jh][0:64, i, :],
                    in_=fnT8[2 * jh][i:128:2, :],
                )
                nc.gpsimd.dma_start(
                    out=fnP[jh][64:128, i, :],
                    in_=fnT8[2 * jh + 1][i:128:2, :],
                )

        # per-class scale S/||w_c|| in [128, NCT] grid layout (c=ct*128+p)
        sinv = small.tile([128, NCT], FP, tag="sinv")
        # bf16 accumulator for the partial softmax sums over class tiles
        acc = msmall.tile([128, N], BF, tag="acc")
        nc.vector.memset(acc[:], 0.0)

        def emit_norm_block(g):
            gall, gsz = GRPOFF[g], GRPSZ[g]
            # n2[c] = sum_d wT[d,c]^2 via ones-matmul over squared chunks
            for c0, csz in SUBCH[g]:
                n2p = psumx.tile([1, 512], FP, tag="n2p", bufs=1,
                                 name=f"n2p_{g}_{c0}")
                for j in range(4):
                    jh, i = j // 2, j % 2
                    wsq = work.tile([128, 512], BF, tag="wsq",
                                    name=f"wsq_{g}_{c0}_{j}")
                    nc.vector.tensor_mul(wsq[:, :csz],
                                         wsb[jh][g][:, i, ds(c0, csz)],
                                         wsb[jh][g][:, i, ds(c0, csz)])
                    nc.tensor.matmul(
                        n2p[:, :csz], ones_bf[:], wsq[:, :csz],
                        start=(j == 0), stop=(j == 3),
                    )
                n2c = work.tile([1, 512], FP, tag="n2c",
                                name=f"n2c_{g}_{c0}")
                nc.vector.tensor_copy(n2c[:, :csz], n2p[:, :csz])
                nc.gpsimd.dma_start(out=n2_d.ap()[:, ds(gall + c0, csz)],
                                    in_=n2c[:, :csz])

            # sinv = S * rsqrt(n2 + eps) in grid layout (DVE only)
            gcols = gsz // 128
            gw0 = gall // 128
            n2g = work.tile([128, 20], FP, tag="n2g", name=f"n2g_{g}")
            nc.gpsimd.dma_start(out=n2g[:, :gcols],
                                in_=n2_grid[:, ds(gw0, gcols)])
            # guard pad columns (n2 == 0): +1e-24 keeps rsqrt finite
            nc.vector.tensor_scalar_add(n2g[:, :gcols], n2g[:, :gcols], 1e-24)
            invw = dve_rsqrt(nc, work, n2g[:, :gcols], 128, gcols,
                             "rsg", iters=1)
            nc.vector.tensor_scalar_mul(sinv[:, ds(gw0, gcols)], invw, S / 16.0)

        def emit_main(g):
            gall, gsz = GRPOFF[g], GRPSZ[g]
            for lc in range(gsz // 128):
                ct = gall // 128 + lc
                z = zpsum.tile([128, N], FP, tag="z", name=f"z_{ct}")
                for jh in range(2):
                    for h in range(0, N, 512):
                        nc.tensor.matmul(
                            z[:, ds(h, 512)],
                            wsb[jh][g][:, :, ts(lc, 128)],
                            fnP[jh][:, :, ds(h, 512)],
                            start=(jh == 0), stop=(jh == 1),
                            perf_mode=mybir.MatmulPerfMode.DoubleRow,
                        )
                pd = work.tile([128, N], BF, tag="pd", name=f"pd_{ct}")
                nc.scalar.activation(
                    pd[:], z[:], AF.Exp,
                    bias=FB, scale=sinv[:, ts(ct, 1)],
                )
                nc.vector.tensor_add(acc[:], acc[:], pd[:])

        def emit_label_path():
            # ---- label-margin path (redundant on every core) ----
            cosl = small.tile([128, NT], FP, tag="cosl")
            wsql = small.tile([128, NT], FP, tag="wsql")
            for t in range(NT):
                nc.vector.scalar_tensor_tensor(
                    out=dump[:], in0=wlts[t][:], scalar=1.0, in1=wlts[t][:],
                    op0=OP.mult, op1=OP.mult,
                    accum_out=wsql[:, ts(t, 1)],
                )
            winv = dve_rsqrt(nc, small, wsql[:], 128, NT, "rsw", iters=2)
            for t in range(NT):
                wln_t = work.tile([128, D], FP, tag="wln_t")
                nc.vector.tensor_scalar_mul(wln_t[:], wlts[t][:],
                                            winv[:, ts(t, 1)])
                nc.vector.scalar_tensor_tensor(
                    out=dump[:], in0=fn32[t][:], scalar=1.0, in1=wln_t[:],
                    op0=OP.mult, op1=OP.mult,
                    accum_out=cosl[:, ts(t, 1)],
                )

            # margin math on [128, 8]
            nc.vector.tensor_scalar(cosl[:], cosl[:], -1.0, 1.0, OP.max, OP.min)
            sq = small.tile([128, NT], FP, tag="sq")
            nc.vector.tensor_mul(sq[:], cosl[:], cosl[:])
            sin2 = small.tile([128, NT], FP, tag="sin2")
            nc.vector.tensor_scalar(sin2[:], sq[:], -1.0, 1.0 + 1e-5,
                                    OP.mult, OP.add)
            # sin = sin2 * rsqrt(sin2)  (DVE only)
            rs2 = dve_rsqrt(nc, small, sin2[:], 128, NT, "rss", iters=2)
            sinl = small.tile([128, NT], FP, tag="sinl")
            nc.vector.tensor_mul(sinl[:], sin2[:], rs2)
            cosm = small.tile([128, NT], FP, tag="cosm")
            sinm = small.tile([128, NT], FP, tag="sinm")
            nc.vector.tensor_scalar_mul(sinm[:], sinl[:], SIN_M)
            nc.vector.scalar_tensor_tensor(
                out=cosm[:], in0=cosl[:], scalar=COS_M, in1=sinm[:],
                op0=OP.mult, op1=OP.subtract,
            )
            other = small.tile([128, NT], FP, tag="other")
            nc.vector.scalar_tensor_tensor(
                out=other[:], in0=sinl[:], scalar=-MARGIN, in1=cosl[:],
                op0=OP.mult, op1=OP.add,
            )
            mask = small.tile([128, NT], mybir.dt.uint8, tag="mask")
            nc.vector.tensor_single_scalar(mask[:], cosl[:], MIN_COS, OP.is_gt)
            target = small.tile([128, NT], FP, tag="target")
            nc.vector.select(target[:], mask[:], cosm[:], other[:])
            tlog = small.tile([128, NT], FP, tag="tlog")
            nc.vector.tensor_scalar_mul(tlog[:], target[:], S)
            e1 = small.tile([128, NT], FP, tag="e1")
            nc.scalar.activation(e1[:], target[:], AF.Exp, bias=FB, scale=S)
            e2 = small.tile([128, NT], FP, tag="e2")
            nc.scalar.activation(e2[:], cosl[:], AF.Exp, bias=FB, scale=S)
            delta = small.tile([128, NT], FP, tag="delta")
            nc.vector.tensor_sub(delta[:], e1[:], e2[:])

            return tlog, delta


        # pipelined emission: norm chains two groups ahead of the mains
        emit_norm_block(0)
        emit_norm_block(1)
        for g in range(NGRP):
            if g + 2 <= NGRP - 1:
                emit_norm_block(g + 2)
            if g == NGRP - 2:
                # warm up the collective path late in the run: absorbs
                # the ncfw first-collective setup AND re-aligns core
                # skew shortly before the real all-reduce
                warm = constp.tile([128, 1], FP, tag="warm")
                nc.vector.memset(warm[:], 0.0)
                nc.sync.dma_start(out=ccw_in.ap(), in_=warm[:])
                nc.gpsimd.collective_compute(
                    "AllReduce",
                    OP.add,
                    replica_groups=[list(range(NCORES))],
                    ins=[ccw_in.ap().opt()],
                    outs=[ccw_out.ap().opt()],
                )
            emit_main(g)
            if g == 0:
                tlog, delta = emit_label_path()

        # ---- reduce over class partitions, all-reduce, finish loss ----
        psum_rows = [
            psumx.tile([1, 512], FP, tag=("tp" if h == 0 else "n2p"),
                       name=f"rowp{h}")
            for h in range(2)
        ]
        for h in range(2):
            nc.tensor.matmul(psum_rows[h][:], ones_bf[:],
                             acc[:, ds(h * 512, 512)], start=True, stop=True)
        prow = msmall.tile([1, N], FP, tag="prow")
        for h in range(2):
            nc.vector.tensor_copy(prow[:, ds(h * 512, 512)],
                                  psum_rows[h][:])
        nc.sync.dma_start(out=cc_in.ap(), in_=prow[:])
        nc.gpsimd.collective_compute(
            "AllReduce",
            OP.add,
            replica_groups=[list(range(NCORES))],
            ins=[cc_in.ap().opt()],
            outs=[cc_out.ap().opt()],
        )
        pg = msmall.tile([128, NT], FP, tag="pg")
        nc.gpsimd.dma_start(out=pg[:], in_=ccout_grid)

        u = msmall.tile([128, NT], FP, tag="u")
        nc.vector.tensor_add(u[:], pg[:], delta[:])
        # HW Ln is inaccurate for tiny args; rescale by an exact 2^70
        lnu = msmall.tile([128, NT], FP, tag="lnu")
        nc.scalar.activation(lnu[:], u[:], AF.Ln, bias=0.0, scale=2.0 ** 70)
        nll = msmall.tile([128, NT], FP, tag="nll")
        nc.vector.scalar_tensor_tensor(
            out=nll[:], in0=lnu[:], scalar=(-FB - 70.0 * LN2), in1=tlog[:],
            op0=OP.add, op1=OP.subtract,
        )
        nsum = msmall.tile([128, 1], FP, tag="nsum")
        nc.vector.tensor_reduce(nsum[:], nll[:],
                                axis=mybir.AxisListType.X, op=OP.add)
        ones_fp2 = msmall.tile([128, 1], FP, tag="ones_fp2")
        nc.vector.memset(ones_fp2[:], 1.0)
        lp = psumx.tile([1, 1], FP, tag="tp")
        nc.tensor.matmul(lp[:], ones_fp2[:], nsum[:], start=True, stop=True)
        res = msmall.tile([1, 1], FP, tag="res")
        nc.scalar.activation(res[:], lp[:], AF.Copy, bias=0.0, scale=1.0 / N)
        nc.sync.dma_start(out=out_d.ap(), in_=res[:])

    nc.compile()
    return nc


_NC_CACHE = None


def _get_nc():
    global _NC_CACHE
    if _NC_CACHE is None:
        _NC_CACHE = build_nc()
    return _NC_CACHE


def _make_in_maps(feats, w, labels):
    feats = np.asarray(feats, dtype=np.float32).reshape(N, D)
    w = np.asarray(w, dtype=np.float32)
    labels = np.asarray(labels).astype(np.int64)
    wl = np.ascontiguousarray(w[labels]).astype(np.float32)
    in_maps = []
    for i in range(NCORES):
        # packed fp8 weights: wt[jh, p, q, c] = 8 * w[c, jh*256 + 2p + q]
        wpad = np.zeros((CP, D), dtype=np.float32)
        wpad[:CS] = w[i * CS:(i + 1) * CS] * 8.0
        wq = np.empty((2, 128, 2, CP), dtype=ml_dtypes.float8_e4m3)
        wT = wpad.T  # [D, CP]
        for jh in range(2):
            blk = wT[jh * 256:(jh + 1) * 256]          # [256, CP]
            wq[jh] = blk.reshape(128, 2, CP).astype(ml_dtypes.float8_e4m3)
        in_maps.append({"wt": wq, "feats": feats, "wl": wl})
    return in_maps


def run(feats, w, labels, trace=False):
    nc = _get_nc()
    in_maps = _make_in_maps(feats, w, labels)
    res = run_bass_kernel_spmd(nc, in_maps, core_ids=list(range(NCORES)),
                               trace=trace)
    out = np.asarray(res.results[0]["out"], dtype=np.float32).reshape(())
    return out, res


def kernel(feats, w, labels):
    out, _ = run(feats, w, labels)
    return out
